# revision 20
# baseline (speedup 1.0000x reference)
"""CenterLoss kernel for Trainium2 (8 NeuronCores, data-parallel over batch).

reference: mean(clip(distmat[i, labels[i]])) where
  distmat[i,c] = ||x_i||^2 + ||c_c||^2 - 2 x_i . c_c
i.e. the loss only needs dist_i = ||x_i - centers[labels[i]]||^2 — a gather +
elementwise + reduce; the full (N, C) matmul in the reference is dead work.
The clip is provably inactive for this problem (distances are O(1e3), far from
1e-12/1e12), so the mean only needs per-partition sums, not per-row values.

Per core (512 rows of the 4096-row batch), all data in fp16 (verified ~1e-4
rel err on the actual seeded inputs vs the 2e-2 gate; halves HBM traffic):

  - 4x 1-column indirect-DMA gathers (128 center rows each). Constraints
    found the hard way:
      * walrus's indirect path consumes ONE offset per partition; multi-
        column gathers silently fetch contiguous rows (diverges from
        CoreSim). So 128 rows per op is the max.
      * the offset AP must start at its tile base: sliced offset APs
        mis-address for non-4B dest dtypes -> one label tile per gather.
      * InstDMAGatherAnt (MoE path) gathers all 512 in one op but triggers
        a ~8us Q7 library reload + ~4.7us emission — slower overall.
  - 3-term accumulation sum(x^2) + sum(c^2) - 2*sum(x*c), pipelined per
    gather chunk as each completion sem fires:
      ACT (1.2GHz, 1 elem/cyc/lane): Square(x-slice) early + Square(c_i)
      DVE (0.96GHz, 1 elem/cyc/lane): rest of x^2 early + x*c_i
  - accumulator columns DMA out per core; host sums and divides by N.
"""

import os

import numpy as np

# clears a wedged NeuronCore from a previous crashed run at NRT init
os.environ.setdefault("NEURON_RT_RESET_CORES", "1")

N, D, C = 4096, 512, 10000
NCORES = 8
ROWS_PER_CORE = N // NCORES  # 512
P = 128
J = ROWS_PER_CORE // P  # 4 rows (columns of D) per partition

_cache = {}

# ---- tuning knobs -----------------------------------------------------------
CHUNKS_X = [2, 2]  # columns per x-load DMA (all on scalar: sync holds labels)
XSQ_ACT_COLS = 2  # of the J=4 x columns, how many ACT squares (DVE takes rest)
SCRATCH_SIZE = 65536  # SWDGE descriptor ring
# -----------------------------------------------------------------------------


def _build_nc():
    import concourse.bass as bass
    import concourse.mybir as mybir
    from concourse import bacc
    from concourse.tile import TileContext

    assert sum(CHUNKS_X) == J

    nc = bacc.Bacc(
        "TRN2",
        target_bir_lowering=False,
        debug=False,
        num_devices=NCORES,
        dynamic_dma_scratch_size=SCRATCH_SIZE,
    )
    fp16 = mybir.dt.float16
    x = nc.dram_tensor("x", [P, J * D], fp16, kind="ExternalInput")
    labels = nc.dram_tensor("labels", [P, J], mybir.dt.int32, kind="ExternalInput")
    centers = nc.dram_tensor("centers", [C, D], fp16, kind="ExternalInput")
    out = nc.dram_tensor("out", [P, 2 + 2 * J], mybir.dt.float32, kind="ExternalOutput")

    with TileContext(nc) as tc:
        with (
            tc.tile_pool(name="io", bufs=1) as io_pool,
            tc.tile_pool(name="work", bufs=1) as work,
        ):
            # label tiles first on sync — the gathers are gated on them.
            # One tile per gather: offset APs must start at a tile base.
            lab_tiles = []
            for j in range(J):
                lt = io_pool.tile([P, 1], mybir.dt.int32, tag=f"lab{j}")
                nc.sync.dma_start(out=lt[:], in_=labels[:, j : j + 1])
                lab_tiles.append(lt)

            # x tile loaded in chunks split across the two HWDGE queues
            xt = io_pool.tile([P, J * D], fp16, tag="x")
            hw_engs = [nc.scalar, nc.scalar]
            col0 = 0
            for gi, cols in enumerate(CHUNKS_X):
                hw_engs[gi % 2].dma_start(
                    out=xt[:, col0 * D : (col0 + cols) * D],
                    in_=x[:, col0 * D : (col0 + cols) * D],
                )
                col0 += cols

            # cols: 0=xsq_act 1=xsq_dve, 2+j=csq_j, 2+J+j=xc_j (accum_out
            # ASSIGNS the op's own sum — every op needs its own column)
            acc = io_pool.tile([P, 2 + 2 * J], mybir.dt.float32, tag="acc")

            # 4x 128-row gathers, back-to-back SWDGE emissions
            gts = []
            for j in range(J):
                gt = io_pool.tile([P, D], fp16, tag=f"g{j}")
                gts.append(gt)
                nc.gpsimd.indirect_dma_start(
                    out=gt[:],
                    out_offset=None,
                    in_=centers[:],
                    in_offset=bass.IndirectOffsetOnAxis(ap=lab_tiles[j][:], axis=0),
                )

            # sum(x^2): ACT takes XSQ_ACT_COLS columns, DVE the rest — both
            # run while the gathers stream in
            a = XSQ_ACT_COLS
            if a > 0:
                sq = work.tile([P, a * D], fp16, tag="wxa")
                nc.scalar.activation(
                    out=sq[:],
                    in_=xt[:, : a * D],
                    func=mybir.ActivationFunctionType.Square,
                    accum_out=acc[:, 0:1],
                )
            if a < J:
                sq = work.tile([P, (J - a) * D], fp16, tag="wxv")
                nc.vector.scalar_tensor_tensor(
                    out=sq[:],
                    in0=xt[:, a * D :],
                    scalar=0.0,
                    in1=xt[:, a * D :],
                    op0=mybir.AluOpType.add,
                    op1=mybir.AluOpType.mult,
                    accum_out=acc[:, 1:2],
                )

            # as each gather lands: c^2 on ACT, x*c on DVE
            for j, gt in enumerate(gts):
                sq = work.tile([P, D], fp16, tag=f"wca{j}")
                nc.scalar.activation(
                    out=sq[:],
                    in_=gt[:],
                    func=mybir.ActivationFunctionType.Square,
                    accum_out=acc[:, 2 + j : 3 + j],
                )
                xc = work.tile([P, D], fp16, tag=f"wxc{j}")
                nc.vector.scalar_tensor_tensor(
                    out=xc[:],
                    in0=xt[:, j * D : (j + 1) * D],
                    scalar=0.0,
                    in1=gt[:],
                    op0=mybir.AluOpType.add,
                    op1=mybir.AluOpType.mult,
                    accum_out=acc[:, 2 + J + j : 3 + J + j],
                )

            nc.sync.dma_start(out=out[:], in_=acc[:])

    nc.compile()
    return nc


def _run(in_maps, trace=False):
    from concourse.bass_utils import run_bass_kernel_spmd

    if "nc" not in _cache:
        _cache["nc"] = _build_nc()
    return run_bass_kernel_spmd(
        _cache["nc"], in_maps, list(range(NCORES)), trace=trace
    )


def kernel(x, labels, centers, _trace=False):
    x = np.asarray(x, dtype=np.float32).astype(np.float16)
    labels = np.asarray(labels).astype(np.int32)
    centers = np.ascontiguousarray(
        np.asarray(centers, dtype=np.float32).astype(np.float16)
    )

    R = ROWS_PER_CORE
    in_maps = []
    for c in range(NCORES):
        lo = c * R
        hi = lo + R
        in_maps.append(
            {
                "x": np.ascontiguousarray(x[lo:hi].reshape(P, J * D)),
                "labels": np.ascontiguousarray(labels[lo:hi].reshape(P, J)),
                "centers": centers,
            }
        )

    res = _run(in_maps, trace=_trace)
    total = 0.0
    for c in range(NCORES):
        a = np.asarray(res.results[c]["out"], dtype=np.float64)  # [P, 2+2J]
        total += a[:, : 2 + J].sum() - 2.0 * a[:, 2 + J :].sum()
    # the clip is inactive for these inputs (dist >> 1e-12), so mean(clip(d))
    # == sum(d)/N
    loss = total / N
    out = np.asarray(loss, dtype=np.float32)
    if _trace:
        return out, res
    return out


# revision 23
# speedup vs baseline: 1.0180x; 1.0180x over previous
"""CenterLoss kernel for Trainium2 (8 NeuronCores, data-parallel over batch).

reference: mean(clip(distmat[i, labels[i]])) where
  distmat[i,c] = ||x_i||^2 + ||c_c||^2 - 2 x_i . c_c
i.e. the loss only needs dist_i = ||x_i - centers[labels[i]]||^2 — a gather +
elementwise + reduce; the full (N, C) matmul in the reference is dead work.
The clip is provably inactive for this problem (distances are O(1e3), far from
1e-12/1e12), so the mean only needs per-partition sums, not per-row values.

Per core (512 rows of the 4096-row batch), all data in fp16 (verified ~1e-4
rel err on the actual seeded inputs vs the 2e-2 gate; halves HBM traffic):

  - 4x 1-column indirect-DMA gathers (128 center rows each). Constraints
    found the hard way:
      * walrus's indirect path consumes ONE offset per partition; multi-
        column gathers silently fetch contiguous rows (diverges from
        CoreSim). So 128 rows per op is the max.
      * the offset AP must start at its tile base: sliced offset APs
        mis-address for non-4B dest dtypes -> one label tile per gather.
      * InstDMAGatherAnt (MoE path) gathers all 512 in one op but triggers
        a ~8us Q7 library reload + ~4.7us emission — slower overall.
  - 3-term accumulation sum(x^2) + sum(c^2) - 2*sum(x*c), pipelined per
    gather chunk as each completion sem fires:
      ACT (1.2GHz, 1 elem/cyc/lane): Square(x-slice) early + Square(c_i)
      DVE (0.96GHz, 1 elem/cyc/lane): rest of x^2 early + x*c_i
  - accumulator columns DMA out per core; host sums and divides by N.
"""

import os

import numpy as np

# clears a wedged NeuronCore from a previous crashed run at NRT init
os.environ.setdefault("NEURON_RT_RESET_CORES", "1")

N, D, C = 4096, 512, 10000
NCORES = 8
ROWS_PER_CORE = N // NCORES  # 512
P = 128
J = ROWS_PER_CORE // P  # 4 rows (columns of D) per partition

_cache = {}

# ---- tuning knobs -----------------------------------------------------------
CHUNKS_X = [2, 2]  # columns per x-load DMA (all on scalar: sync holds labels)
XSQ_ACT_COLS = 2  # of the J=4 x columns, how many ACT squares (DVE takes rest)
SCRATCH_SIZE = 65536  # SWDGE descriptor ring
# -----------------------------------------------------------------------------


def _build_nc():
    import concourse.bass as bass
    import concourse.mybir as mybir
    from concourse import bacc
    from concourse.tile import TileContext

    assert sum(CHUNKS_X) == J

    nc = bacc.Bacc(
        "TRN2",
        target_bir_lowering=False,
        debug=False,
        num_devices=NCORES,
        dynamic_dma_scratch_size=SCRATCH_SIZE,
    )
    fp16 = mybir.dt.float16
    x = nc.dram_tensor("x", [P, J * D], fp16, kind="ExternalInput")
    labels = nc.dram_tensor("labels", [P, J], mybir.dt.int32, kind="ExternalInput")
    centers = nc.dram_tensor("centers", [C, D], fp16, kind="ExternalInput")
    out = nc.dram_tensor("out", [P, 2 + 2 * J], mybir.dt.float32, kind="ExternalOutput")

    with TileContext(nc) as tc:
        with (
            tc.tile_pool(name="io", bufs=1) as io_pool,
            tc.tile_pool(name="work", bufs=1) as work,
        ):
            # labels land in ONE DMA (4 tiny [P,1] DMAs cost ~0.6us each in
            # dispatch+drain); gather offsets must sit at a tile BASE (sliced
            # offset APs mis-address for fp16 dests), so gather 0 reads the
            # loaded tile's base column directly and DVE fans out copies of
            # columns 1-3 into their own tiles (engine-op deps, no DMA
            # receipt on the critical path).
            lab4 = io_pool.tile([P, J], mybir.dt.int32, tag="lab4")
            nc.sync.dma_start(out=lab4[:], in_=labels[:])
            lab_aps = [lab4[:, 0:1]]
            for j in range(1, J):
                lt = io_pool.tile([P, 1], mybir.dt.int32, tag=f"lab{j}")
                nc.vector.tensor_scalar_add(out=lt[:], in0=lab4[:, j : j + 1], scalar1=0)
                lab_aps.append(lt[:])

            # x tile loaded in chunks split across the two HWDGE queues
            xt = io_pool.tile([P, J * D], fp16, tag="x")
            hw_engs = [nc.scalar, nc.scalar]
            col0 = 0
            for gi, cols in enumerate(CHUNKS_X):
                hw_engs[gi % 2].dma_start(
                    out=xt[:, col0 * D : (col0 + cols) * D],
                    in_=x[:, col0 * D : (col0 + cols) * D],
                )
                col0 += cols

            # cols: 0=xsq_act 1=xsq_dve, 2+j=csq_j, 2+J+j=xc_j (accum_out
            # ASSIGNS the op's own sum — every op needs its own column)
            acc = io_pool.tile([P, 2 + 2 * J], mybir.dt.float32, tag="acc")

            # 4x 128-row gathers, back-to-back SWDGE emissions
            gts = []
            for j in range(J):
                gt = io_pool.tile([P, D], fp16, tag=f"g{j}")
                gts.append(gt)
                nc.gpsimd.indirect_dma_start(
                    out=gt[:],
                    out_offset=None,
                    in_=centers[:],
                    in_offset=bass.IndirectOffsetOnAxis(ap=lab_aps[j], axis=0),
                )

            # sum(x^2): ACT takes XSQ_ACT_COLS columns, DVE the rest — both
            # run while the gathers stream in
            a = XSQ_ACT_COLS
            if a > 0:
                sq = work.tile([P, a * D], fp16, tag="wxa")
                nc.scalar.activation(
                    out=sq[:],
                    in_=xt[:, : a * D],
                    func=mybir.ActivationFunctionType.Square,
                    accum_out=acc[:, 0:1],
                )
            if a < J:
                sq = work.tile([P, (J - a) * D], fp16, tag="wxv")
                nc.vector.scalar_tensor_tensor(
                    out=sq[:],
                    in0=xt[:, a * D :],
                    scalar=0.0,
                    in1=xt[:, a * D :],
                    op0=mybir.AluOpType.add,
                    op1=mybir.AluOpType.mult,
                    accum_out=acc[:, 1:2],
                )

            # as each gather lands: c^2 on ACT, x*c on DVE
            for j, gt in enumerate(gts):
                sq = work.tile([P, D], fp16, tag=f"wca{j}")
                nc.scalar.activation(
                    out=sq[:],
                    in_=gt[:],
                    func=mybir.ActivationFunctionType.Square,
                    accum_out=acc[:, 2 + j : 3 + j],
                )
                xc = work.tile([P, D], fp16, tag=f"wxc{j}")
                nc.vector.scalar_tensor_tensor(
                    out=xc[:],
                    in0=xt[:, j * D : (j + 1) * D],
                    scalar=0.0,
                    in1=gt[:],
                    op0=mybir.AluOpType.add,
                    op1=mybir.AluOpType.mult,
                    accum_out=acc[:, 2 + J + j : 3 + J + j],
                )

            nc.sync.dma_start(out=out[:], in_=acc[:])

    nc.compile()
    return nc


def _run(in_maps, trace=False):
    from concourse.bass_utils import run_bass_kernel_spmd

    if "nc" not in _cache:
        _cache["nc"] = _build_nc()
    return run_bass_kernel_spmd(
        _cache["nc"], in_maps, list(range(NCORES)), trace=trace
    )


def kernel(x, labels, centers, _trace=False):
    x = np.asarray(x, dtype=np.float32).astype(np.float16)
    labels = np.asarray(labels).astype(np.int32)
    centers = np.ascontiguousarray(
        np.asarray(centers, dtype=np.float32).astype(np.float16)
    )

    R = ROWS_PER_CORE
    in_maps = []
    for c in range(NCORES):
        lo = c * R
        hi = lo + R
        in_maps.append(
            {
                "x": np.ascontiguousarray(x[lo:hi].reshape(P, J * D)),
                "labels": np.ascontiguousarray(labels[lo:hi].reshape(P, J)),
                "centers": centers,
            }
        )

    res = _run(in_maps, trace=_trace)
    total = 0.0
    for c in range(NCORES):
        a = np.asarray(res.results[c]["out"], dtype=np.float64)  # [P, 2+2J]
        total += a[:, : 2 + J].sum() - 2.0 * a[:, 2 + J :].sum()
    # the clip is inactive for these inputs (dist >> 1e-12), so mean(clip(d))
    # == sum(d)/N
    loss = total / N
    out = np.asarray(loss, dtype=np.float32)
    if _trace:
        return out, res
    return out
